# revision 21
# baseline (speedup 1.0000x reference)
"""Cascaded-attention GRU recurrence on 8 NeuronCores (Bass/Tile).

Problem: B=128, T=75, D=512, V=28. Data-parallel over batch: 16 batch rows
per core, weights replicated. Per-core recurrence over 75 steps with two
independent batch half-groups (8 rows each) pipelined against each other.

Key layout choices (per core, BL=16 local batch):
- d-on-partition layout for the big tensors: UaH' = x@Ua + Ba1 + Ba2 stored
  [128(d-chunk), 4(chunk), 16(b), 75(t)]; per-step tanh runs as one ACT
  instruction per half-group.
- WaS bias (state@Wa, changes per step) broadcast over t via a stride-0 AP
  on the DVE tensor_tensor add.
- scores contracted against Va via PE with a column-replicated Va (VaSEL) so
  the result lands partition-major-ish as REP[75, (b,t)] (all rows equal);
  the free->partition transpose of the softmax weights is done by masking
  REP with a constant diagonal and free-reducing (DVE), yielding
  scoresT[75(t), b].
- softmax normalization deferred: unnormalized exp(scoresT) drives
  block-diag matmuls against XKC = x@[gru_kernel|Co] (precomputed on
  device), producing xm/CoC directly ([8, 112] per group); 1/Z folded into
  the GRU gate/output scalar ops.
- sigmoid via tanh (same ACT table set as exp): sigmoid(a) = .5 + .5*tanh(a/2),
  with gru_rec_kernel pre-halved on host so gate args come out right.
- embedding lookup: softmax probs cast to int32 are 0 unless pred == 1.0, so
  emb[idx]@Wo == w0 + (w1-w0)*[pred >= 1], w = emb@Wo (exact).
- Ba3 dropped (softmax shift invariance). gru_bias[1] folded into the hm
  matmul; gru_bias[0] assumed zero (asserted) as in the problem setup.
"""

import numpy as np

B, T, D, V = 128, 75, 512, 28
NCORES = 8
BL = B // NCORES        # 16 batch rows per core
G = 2                   # half-groups per core
BG = BL // G            # 8 rows per group
SUB = 4                 # softmax sub-group (psum-bank limited: 4*75=300 f32)
NC_, CH = 128, D // 128  # partitions, d-chunks


def _build(nc, tc, tile, bass, mybir, gru_b0_nonzero, steps=T):
    f32 = mybir.dt.float32
    Act = mybir.ActivationFunctionType
    Op = mybir.AluOpType

    # ---------------- DRAM I/O ----------------
    dr = {}
    def din(name, shape):
        dr[name] = nc.dram_tensor(name, shape, f32, kind="ExternalInput")
        return dr[name]

    x_dmaj = din("x_dmaj", [NC_, CH, BL, T])
    ua_k = din("ua_k", [NC_, CH, CH, 128])
    ba12 = din("ba12", [NC_, CH])
    wa = din("wa", [V, D])
    vasel = din("vasel", [NC_, CH, T])
    w2 = din("w2", [NC_, CH, 112])
    wrec_h = din("wrec_h", [V, 84])        # 0.5 * gru_rec_kernel
    b1gru = din("b1gru", [1, 84])          # gru_bias[1] (folded into hm)
    uo = din("uo", [V, V])
    diag = din("diag", [T, T])
    twos75 = din("twos75", [T, 1])
    ones1x16 = din("ones1x16", [1, BL])
    i16 = din("i16", [BL, BL])
    bo2rep = din("bo2rep", [BG, V])        # Bo + w0, replicated rows
    dwrep = din("dwrep", [BG, 1])          # w1-w0, per-partition scalar
    if gru_b0_nonzero:
        b0rep = din("b0rep", [BG, 84])
    y_out = nc.dram_tensor("y", [BL, T, V], f32, kind="ExternalOutput")

    import contextlib
    ctx = contextlib.ExitStack()
    with ctx:
        cst = ctx.enter_context(tc.tile_pool(name="cst", bufs=1))
        wrk = ctx.enter_context(tc.tile_pool(name="wrk", bufs=2))
        wrk3 = ctx.enter_context(tc.tile_pool(name="wrk3", bufs=3))
        pwast = ctx.enter_context(tc.tile_pool(name="pwast", bufs=2, space="PSUM"))
        pbt = ctx.enter_context(tc.tile_pool(name="pbt", bufs=2, space="PSUM"))
        prep = ctx.enter_context(tc.tile_pool(name="prep", bufs=2, space="PSUM"))
        pxm = ctx.enter_context(tc.tile_pool(name="pxm", bufs=2, space="PSUM"))

        # ---------------- constants into SBUF ----------------
        t_x = cst.tile([NC_, CH, BL, T], f32, tag="t_x")
        t_ua = cst.tile([NC_, CH, CH, 128], f32, tag="t_ua")
        t_ba12 = cst.tile([NC_, CH], f32, tag="t_ba12")
        t_wa = cst.tile([V, D], f32, tag="t_wa")
        t_vas = cst.tile([NC_, CH, T], f32, tag="t_vas")
        t_w2 = cst.tile([NC_, CH, 112], f32, tag="t_w2")
        t_wrec = cst.tile([V, 84], f32, tag="t_wrec")
        t_b1 = cst.tile([1, 84], f32, tag="t_b1")
        t_uo = cst.tile([V, V], f32, tag="t_uo")
        t_diag = cst.tile([T, T], f32, tag="t_diag")
        t_two = cst.tile([T, 1], f32, tag="t_two")
        t_o116 = cst.tile([1, BL], f32, tag="t_o116")
        t_i16 = cst.tile([BL, BL], f32, tag="t_i16")
        t_bo2 = cst.tile([BG, V], f32, tag="t_bo2")
        t_dw = cst.tile([BG, 1], f32, tag="t_dw")
        for tt, d_ in [(t_x, x_dmaj), (t_ua, ua_k), (t_ba12, ba12), (t_wa, wa),
                       (t_vas, vasel), (t_w2, w2), (t_wrec, wrec_h), (t_b1, b1gru),
                       (t_uo, uo), (t_diag, diag), (t_two, twos75), (t_o116, ones1x16),
                       (t_i16, i16), (t_bo2, bo2rep), (t_dw, dwrep)]:
            nc.sync.dma_start(tt[:], d_[:])
        if gru_b0_nonzero:
            t_b0 = cst.tile([BG, 84], f32, tag="t_b0")
            nc.sync.dma_start(t_b0[:], b0rep[:])

        # persistent state/work tiles
        t_uahp = cst.tile([NC_, CH, BL, T], f32, tag="t_uahp")     # x@Ua + Ba1 + Ba2
        t_xkc = cst.tile([T, BL, 112], f32, tag="t_xkc")           # x@[gruK|Co] per b
        t_smb = cst.tile([T, BL * BL], f32, tag="t_smb")           # block-diag exp(scoresT)
        t_out = []
        for g in range(G):
            t_out_g = cst.tile([BG, T, V], f32, tag=f"t_out{g}")
            t_out.append(t_out_g)
        nc.vector.memset(t_smb[:], 0.0)

        # ---------------- preamble: UaH' = x@Ua + (Ba1+Ba2) ----------------
        NSL, SL = 3, 400  # bt slices per e-chunk
        for ec in range(CH):
            uah_flat = t_uahp[:, ec, :, :].rearrange("p b t -> p (b t)")
            for i in range(NSL):
                ps = prep.tile([NC_, SL], f32, tag="prep")
                for dc in range(CH):
                    x_sl = t_x[:, dc, :, :].rearrange("p b t -> p (b t)")[
                        :, i * SL:(i + 1) * SL]
                    nc.tensor.matmul(ps[:], t_ua[:, dc, ec, :], x_sl,
                                     start=(dc == 0), stop=(dc == CH - 1))
                nc.scalar.activation(uah_flat[:, i * SL:(i + 1) * SL], ps[:],
                                     Act.Identity, bias=t_ba12[:, ec:ec + 1],
                                     scale=1.0)

        # ---------------- preamble: XKC = x@[gruK|Co] ----------------
        for b in range(BL):
            ps = pxm.tile([T, 112], f32, tag="pxm")
            for dc in range(CH):
                nc.tensor.matmul(ps[:], t_x[:, dc, b, :], t_w2[:, dc, :],
                                 start=(dc == 0), stop=(dc == CH - 1))
            nc.vector.tensor_copy(t_xkc[:, b, :], ps[:])

        # ---------------- recurrent state ----------------
        state = []   # [8,28] b-major per group
        stateT = []  # [28,8] per group
        for g in range(G):
            sg = wrk.tile([BG, V], f32, tag=f"state{g}")
            sTg = wrk.tile([V, BG], f32, tag=f"stateT{g}")
            nc.vector.memset(sg[:], 0.0)
            nc.vector.memset(sTg[:], 0.0)
            state.append(sg)
            stateT.append(sTg)

        # ---------------- the 75 steps ----------------
        for s in range(steps):
            new_state, new_stateT = [], []
            for g in range(G):
                sg, sTg = state[g], stateT[g]
                bs = g * BG
                # --- state-dependent matmuls ---
                wast_ps = pwast.tile([NC_, 40], f32, tag="wastps")
                bt_ps = pbt.tile([BG, 256], f32, tag="btps")
                if s > 0:
                    for c in range(CH):
                        nc.tensor.matmul(wast_ps[:, c * 8:(c + 1) * 8],
                                         t_wa[:, c * 128:(c + 1) * 128], sTg[:],
                                         start=True, stop=True)
                nc.tensor.matmul(bt_ps[:, 0:84], sTg[:], t_wrec[:],
                                 start=True, stop=False)
                nc.tensor.matmul(bt_ps[:, 0:84], t_o116[:, bs:bs + BG], t_b1[:],
                                 start=False, stop=True)
                nc.tensor.matmul(bt_ps[:, 84:112], sTg[:], t_uo[:],
                                 start=True, stop=True)
                hm_sb = wrk.tile([BG, 84], f32, tag=f"hm{g}")
                nc.vector.tensor_copy(hm_sb[:], bt_ps[:, 0:84])

                # --- Y = UaH' + WaS (broadcast over t), tanh ---
                tanhY = wrk.tile([NC_, CH, BG, T], f32, tag=f"tanhY{g}")
                if s > 0:
                    wast_sb = wrk.tile([NC_, 32], f32, tag=f"wastsb{g}")
                    nc.vector.tensor_copy(wast_sb[:], wast_ps[:, 0:32])
                    Yg = wrk.tile([NC_, CH, BG, T], f32, tag=f"Y{g}")
                    for c in range(CH):
                        w_sl = wast_sb[:, c * 8:(c + 1) * 8]
                        w_bc = bass.AP(w_sl.tensor, w_sl.offset,
                                       [list(w_sl.ap[0]), list(w_sl.ap[1]), [0, T]])
                        nc.vector.tensor_tensor(
                            Yg[:, c, :, :],
                            t_uahp[:, c, bs:bs + BG, :], w_bc, Op.add)
                    nc.scalar.activation(tanhY[:], Yg[:], Act.Tanh)
                else:
                    nc.scalar.activation(tanhY[:], t_uahp[:, :, bs:bs + BG, :],
                                         Act.Tanh)

                # --- scores -> scoresT via REP + diag-mask + reduce ---
                scT = wrk.tile([T, BG], f32, tag=f"scT{g}")
                for u in range(BG // SUB):
                    rep_ps = prep.tile([T, SUB * T], f32, tag="prep")
                    for c in range(CH):
                        nc.tensor.matmul(
                            rep_ps[:], t_vas[:, c, :],
                            tanhY[:, c, u * SUB:(u + 1) * SUB, :],
                            start=(c == 0), stop=(c == CH - 1))
                    msk = wrk3.tile([T, SUB, T], f32, tag="msk")
                    d_ap = t_diag[:]
                    d_bc = bass.AP(d_ap.tensor, d_ap.offset,
                                   [list(d_ap.ap[0]), [0, SUB], list(d_ap.ap[1])])
                    rep3 = rep_ps[:].rearrange("p (b t) -> p b t", b=SUB)
                    nc.vector.tensor_tensor(msk[:], rep3, d_bc, Op.mult)
                    nc.vector.tensor_reduce(scT[:, u * SUB:(u + 1) * SUB], msk[:],
                                            mybir.AxisListType.X, Op.add)

                # --- exp, Z, SmBlk ---
                expT = wrk.tile([T, BG], f32, tag=f"expT{g}")
                nc.scalar.activation(expT[:], scT[:], Act.Exp)
                nc.tensor.matmul(bt_ps[:, 112:113], expT[:], t_two[:],
                                 start=True, stop=True)
                rhalf = wrk.tile([BG, 1], f32, tag=f"rhalf{g}")
                nc.vector.reciprocal(rhalf[:], bt_ps[:, 112:113])
                rfull = wrk.tile([BG, 1], f32, tag=f"rfull{g}")
                nc.vector.tensor_scalar(rfull[:], rhalf[:], 2.0, None, Op.mult)
                smb_dst = bass.AP(t_smb.tensor, t_smb[:].offset + 17 * bs,
                                  [list(t_smb[:].ap[0]), [17, BG]])
                nc.vector.tensor_copy(smb_dst, expT[:])

                # --- xm | CoC (unnormalized) ---
                xm_ps = pxm.tile([BG, 112], f32, tag="pxm")
                for bl in range(BG):
                    b = bs + bl
                    nc.tensor.matmul(xm_ps[:], t_smb[:, 16 * b + bs:16 * b + bs + BG],
                                     t_xkc[:, b, :],
                                     start=(bl == 0), stop=(bl == BG - 1))

                # --- GRU gates ---
                zr = wrk.tile([BG, 56], f32, tag=f"zr{g}")
                nc.vector.scalar_tensor_tensor(zr[:], xm_ps[:, 0:56], rhalf[:],
                                               hm_sb[:, 0:56], Op.mult, Op.add)
                if gru_b0_nonzero:
                    nc.vector.tensor_tensor(zr[:], zr[:], t_b0[:, 0:56], Op.add)
                tz = wrk.tile([BG, 56], f32, tag=f"tz{g}")
                nc.scalar.activation(tz[:], zr[:], Act.Tanh)
                s1 = wrk.tile([BG, V], f32, tag=f"s1{g}")
                nc.vector.scalar_tensor_tensor(s1[:], tz[:, V:56], 1.0,
                                               hm_sb[:, 56:84], Op.add, Op.mult)
                ah = wrk.tile([BG, V], f32, tag=f"ah{g}")
                nc.vector.scalar_tensor_tensor(ah[:], xm_ps[:, 56:84], rfull[:],
                                               s1[:], Op.mult, Op.add)
                if gru_b0_nonzero:
                    nc.vector.tensor_tensor(ah[:], ah[:], t_b0[:, 56:84], Op.add)
                hh = wrk.tile([BG, V], f32, tag=f"hh{g}")
                nc.scalar.activation(hh[:], ah[:], Act.Tanh)
                d1 = wrk.tile([BG, V], f32, tag=f"d1{g}")
                nc.vector.tensor_sub(d1[:], sg[:], hh[:])
                d2 = wrk.tile([BG, V], f32, tag=f"d2{g}")
                nc.vector.tensor_add(d2[:], sg[:], hh[:])
                m1 = wrk.tile([BG, V], f32, tag=f"m1{g}")
                nc.vector.tensor_mul(m1[:], tz[:, 0:V], d1[:])
                ns = wrk.tile([BG, V], f32, tag=f"state{g}")
                nc.vector.tensor_tensor(ns[:], m1[:], d2[:], Op.add)
                nc.vector.tensor_scalar(ns[:], ns[:], 0.5, None, Op.mult)

                # --- stateT for next step ---
                nc.tensor.transpose(wast_ps[0:V, 32:40], ns[:], t_i16[0:BG, 0:BG])
                nsT = wrk.tile([V, BG], f32, tag=f"stateT{g}")
                nc.vector.tensor_copy(nsT[:], wast_ps[0:V, 32:40])

                # --- pred logits + softmax (this step's output) ---
                l1 = wrk.tile([BG, V], f32, tag=f"l1{g}")
                if s > 0:
                    ind = wrk.tile([BG, V], f32, tag=f"ind{g}")
                    nc.vector.tensor_scalar(ind[:], t_out[g][:, s - 1, :], 1.0,
                                            None, Op.is_ge)
                    l2 = wrk.tile([BG, V], f32, tag=f"l2{g}")
                    nc.vector.scalar_tensor_tensor(l2[:], ind[:], t_dw[:],
                                                   t_bo2[:], Op.mult, Op.add)
                    nc.vector.scalar_tensor_tensor(
                        l1[:], xm_ps[:, 84:112], rfull[:], l2[:],
                        Op.mult, Op.add)
                else:
                    nc.vector.scalar_tensor_tensor(
                        l1[:], xm_ps[:, 84:112], rfull[:], t_bo2[:],
                        Op.mult, Op.add)
                logits = wrk.tile([BG, V], f32, tag=f"logits{g}")
                nc.vector.tensor_tensor(logits[:], l1[:], bt_ps[:, 84:112],
                                        Op.add)
                expP = wrk.tile([BG, V], f32, tag=f"expP{g}")
                zp = wrk.tile([BG, 1], f32, tag=f"zp{g}")
                nc.scalar.activation(expP[:], logits[:], Act.Exp,
                                     accum_out=zp[:])
                rp = wrk.tile([BG, 1], f32, tag=f"rp{g}")
                nc.vector.reciprocal(rp[:], zp[:])
                nc.vector.tensor_scalar(t_out[g][:, s, :], expP[:], rp[:],
                                        None, Op.mult)

                new_state.append(ns)
                new_stateT.append(nsT)

            state, stateT = new_state, new_stateT

        for g in range(G):
            nc.sync.dma_start(y_out[g * BG:(g + 1) * BG, :, :], t_out[g][:])
    return dr, y_out


_CACHE = {}


def _get_program(gru_b0_nonzero, steps=T):
    key = (bool(gru_b0_nonzero), steps)
    if key in _CACHE:
        return _CACHE[key]
    import concourse.bass as bass
    import concourse.bacc as bacc
    import concourse.tile as tile
    from concourse import mybir

    nc = bacc.Bacc("TRN2", target_bir_lowering=False, debug=False,
                   num_devices=NCORES)
    with tile.TileContext(nc) as tc:
        _build(nc, tc, tile, bass, mybir, gru_b0_nonzero, steps)
    nc.compile()
    _CACHE[key] = nc
    return nc


def _prep_core_inputs(inputs, core):
    x = inputs["x"]
    xs = np.ascontiguousarray(x[core * BL:(core + 1) * BL]).astype(np.float32)
    # [16,75,512] -> [128, chunk, b, t]
    x_dmaj = np.ascontiguousarray(
        xs.reshape(BL, T, CH, 128).transpose(3, 2, 0, 1))
    return x_dmaj


def _prep_weights(inputs):
    f = np.float32
    Ua = inputs["Ua"].astype(f)
    ua_k = np.ascontiguousarray(
        Ua.reshape(CH, 128, CH, 128).transpose(1, 0, 2, 3))
    ba = (inputs["Ba1"] + inputs["Ba2"]).astype(f).reshape(CH, 128)
    ba12 = np.ascontiguousarray(ba.T)
    Va = inputs["Va"].astype(f).reshape(CH, 128)
    vasel = np.ascontiguousarray(
        np.repeat(Va.T[:, :, None], T, axis=2))
    w2 = np.concatenate([inputs["gru_kernel"], inputs["Co"]], axis=1).astype(f)
    w2 = np.ascontiguousarray(w2.reshape(CH, 128, 112).transpose(1, 0, 2))
    w = (inputs["emb"].astype(f) @ inputs["Wo"].astype(f)).reshape(-1)
    w0, w1 = float(w[0]), float(w[1])
    gb = inputs["gru_bias"].astype(f)
    out = {
        "ua_k": ua_k, "ba12": ba12, "wa": inputs["Wa"].astype(f),
        "vasel": vasel, "w2": w2,
        "wrec_h": 0.5 * inputs["gru_rec_kernel"].astype(f),
        "b1gru": gb[1:2], "uo": inputs["Uo"].astype(f),
        "diag": np.eye(T, dtype=f),
        "twos75": np.full([T, 1], 2.0, dtype=f),
        "ones1x16": np.ones([1, BL], dtype=f),
        "i16": np.eye(BL, dtype=f),
        "bo2rep": np.repeat(inputs["Bo"].astype(f) + w0, BG, axis=0),
        "dwrep": np.full([BG, 1], w1 - w0, dtype=f),
    }
    b0 = gb[0]
    if np.any(b0 != 0):
        out["b0rep"] = np.repeat(b0[None, :], BG, axis=0)
    return out, bool(np.any(b0 != 0))


def kernel(**inputs):
    from concourse.bass_utils import run_bass_kernel_spmd

    weights, b0nz = _prep_weights(inputs)
    nc = _get_program(b0nz)
    in_maps = []
    for core in range(NCORES):
        m = dict(weights)
        m["x_dmaj"] = _prep_core_inputs(inputs, core)
        in_maps.append(m)
    res = run_bass_kernel_spmd(nc, in_maps, core_ids=list(range(NCORES)))
    out = np.concatenate([res.results[c]["y"] for c in range(NCORES)], axis=0)
    return out.astype(np.float32)





# revision 24
# speedup vs baseline: 1.0907x; 1.0907x over previous
"""Cascaded-attention GRU recurrence on 8 NeuronCores (Bass/Tile).

Problem: B=128, T=75, D=512, V=28. Data-parallel over batch: 16 batch rows
per core, weights replicated. Per-core recurrence over 75 steps with two
independent batch half-groups (8 rows each) pipelined against each other.

Key layout choices (per core, BL=16 local batch):
- d-on-partition layout for the big tensors: UaH' = x@Ua + Ba1 + Ba2 stored
  [128(d-chunk), 4(chunk), 16(b), 75(t)]; per-step tanh runs as one ACT
  instruction per half-group.
- WaS bias (state@Wa, changes per step) broadcast over t via a stride-0 AP
  on the DVE tensor_tensor add.
- scores contracted against Va via PE with a column-replicated Va (VaSEL) so
  the result lands partition-major-ish as REP[75, (b,t)] (all rows equal);
  the free->partition transpose of the softmax weights is done by masking
  REP with a constant diagonal and free-reducing (DVE), yielding
  scoresT[75(t), b].
- softmax normalization deferred: unnormalized exp(scoresT) drives
  block-diag matmuls against XKC = x@[gru_kernel|Co] (precomputed on
  device), producing xm/CoC directly ([8, 112] per group); 1/Z folded into
  the GRU gate/output scalar ops.
- sigmoid via tanh (same ACT table set as exp): sigmoid(a) = .5 + .5*tanh(a/2),
  with gru_rec_kernel pre-halved on host so gate args come out right.
- embedding lookup: softmax probs cast to int32 are 0 unless pred == 1.0, so
  emb[idx]@Wo == w0 + (w1-w0)*[pred >= 1], w = emb@Wo (exact).
- Ba3 dropped (softmax shift invariance). gru_bias[1] folded into the hm
  matmul; gru_bias[0] assumed zero (asserted) as in the problem setup.
"""

import numpy as np

B, T, D, V = 128, 75, 512, 28
NCORES = 8
BL = B // NCORES        # 16 batch rows per core
G = 2                   # half-groups per core
BG = BL // G            # 8 rows per group
SUBS = [(0, 6), (6, 6), (12, 4)]  # softmax sub-groups (psum bank = 512 f32)
NC_, CH = 128, D // 128  # partitions, d-chunks


def _build(nc, tc, tile, bass, mybir, gru_b0_nonzero, steps=T):
    f32 = mybir.dt.float32
    Act = mybir.ActivationFunctionType
    Op = mybir.AluOpType

    # ---------------- DRAM I/O ----------------
    dr = {}
    def din(name, shape):
        dr[name] = nc.dram_tensor(name, shape, f32, kind="ExternalInput")
        return dr[name]

    x_dmaj = din("x_dmaj", [NC_, CH, BL, T])
    ua_k = din("ua_k", [NC_, CH, CH, 128])
    ba12 = din("ba12", [NC_, CH])
    wa = din("wa", [V, D])
    vasel = din("vasel", [NC_, CH, T])
    w2 = din("w2", [NC_, CH, 112])
    wrec_h = din("wrec_h", [V, 84])        # 0.5 * gru_rec_kernel
    b1gru = din("b1gru", [1, 84])          # gru_bias[1] (folded into hm)
    uo = din("uo", [V, V])
    diag = din("diag", [T, T])
    twos75 = din("twos75", [T, 1])
    ones1x16 = din("ones1x16", [1, BL])
    i16 = din("i16", [BL, BL])
    bo2rep = din("bo2rep", [BG, V])        # Bo + w0, replicated rows
    dwrep = din("dwrep", [BG, 1])          # w1-w0, per-partition scalar
    if gru_b0_nonzero:
        b0rep = din("b0rep", [BG, 84])
    y_out = nc.dram_tensor("y", [BL, T, V], f32, kind="ExternalOutput")

    import contextlib
    ctx = contextlib.ExitStack()
    with ctx:
        cst = ctx.enter_context(tc.tile_pool(name="cst", bufs=1))
        wrk = ctx.enter_context(tc.tile_pool(name="wrk", bufs=2))
        wrk3 = ctx.enter_context(tc.tile_pool(name="wrk3", bufs=3))
        pwast = ctx.enter_context(tc.tile_pool(name="pwast", bufs=2, space="PSUM"))
        pbt = ctx.enter_context(tc.tile_pool(name="pbt", bufs=2, space="PSUM"))
        prep = ctx.enter_context(tc.tile_pool(name="prep", bufs=2, space="PSUM"))
        pxm = ctx.enter_context(tc.tile_pool(name="pxm", bufs=2, space="PSUM"))

        # ---------------- constants into SBUF ----------------
        t_x = cst.tile([NC_, CH, BL, T], f32, tag="t_x")
        t_ua = cst.tile([NC_, CH, CH, 128], f32, tag="t_ua")
        t_ba12 = cst.tile([NC_, CH], f32, tag="t_ba12")
        t_wa = cst.tile([V, D], f32, tag="t_wa")
        t_vas = cst.tile([NC_, CH, T], f32, tag="t_vas")
        t_w2 = cst.tile([NC_, CH, 112], f32, tag="t_w2")
        t_wrec = cst.tile([V, 84], f32, tag="t_wrec")
        t_b1 = cst.tile([1, 84], f32, tag="t_b1")
        t_uo = cst.tile([V, V], f32, tag="t_uo")
        t_diag = cst.tile([T, T], f32, tag="t_diag")
        t_two = cst.tile([T, 1], f32, tag="t_two")
        t_o116 = cst.tile([1, BL], f32, tag="t_o116")
        t_i16 = cst.tile([BL, BL], f32, tag="t_i16")
        t_bo2 = cst.tile([BG, V], f32, tag="t_bo2")
        t_dw = cst.tile([BG, 1], f32, tag="t_dw")
        for tt, d_ in [(t_x, x_dmaj), (t_ua, ua_k), (t_ba12, ba12), (t_wa, wa),
                       (t_vas, vasel), (t_w2, w2), (t_wrec, wrec_h), (t_b1, b1gru),
                       (t_uo, uo), (t_diag, diag), (t_two, twos75), (t_o116, ones1x16),
                       (t_i16, i16), (t_bo2, bo2rep), (t_dw, dwrep)]:
            nc.sync.dma_start(tt[:], d_[:])
        if gru_b0_nonzero:
            t_b0 = cst.tile([BG, 84], f32, tag="t_b0")
            nc.sync.dma_start(t_b0[:], b0rep[:])

        # persistent state/work tiles
        t_uahp = cst.tile([NC_, CH, BL, T], f32, tag="t_uahp")     # x@Ua + Ba1 + Ba2
        t_xkc = cst.tile([T, BL, 112], f32, tag="t_xkc")           # x@[gruK|Co] per b
        t_smb = cst.tile([T, BL * BL], f32, tag="t_smb")           # block-diag exp(scoresT)
        t_out = []
        for g in range(G):
            t_out_g = cst.tile([BG, T, V], f32, tag=f"t_out{g}")
            if steps < T:  # truncated builds (timing/sim only): avoid
                nc.vector.memset(t_out_g[:], 0.0)  # uninit reads at final DMA
            t_out.append(t_out_g)
        nc.vector.memset(t_smb[:], 0.0)

        # ---------------- preamble: UaH' = x@Ua + (Ba1+Ba2) ----------------
        NSL, SL = 3, 400  # bt slices per e-chunk
        for ec in range(CH):
            uah_flat = t_uahp[:, ec, :, :].rearrange("p b t -> p (b t)")
            for i in range(NSL):
                ps = prep.tile([NC_, SL], f32, tag="prep")
                for dc in range(CH):
                    x_sl = t_x[:, dc, :, :].rearrange("p b t -> p (b t)")[
                        :, i * SL:(i + 1) * SL]
                    nc.tensor.matmul(ps[:], t_ua[:, dc, ec, :], x_sl,
                                     start=(dc == 0), stop=(dc == CH - 1))
                nc.scalar.activation(uah_flat[:, i * SL:(i + 1) * SL], ps[:],
                                     Act.Identity, bias=t_ba12[:, ec:ec + 1],
                                     scale=1.0)

        # ---------------- preamble: XKC = x@[gruK|Co] ----------------
        for b in range(BL):
            ps = pxm.tile([T, 112], f32, tag="pxm")
            for dc in range(CH):
                nc.tensor.matmul(ps[:], t_x[:, dc, b, :], t_w2[:, dc, :],
                                 start=(dc == 0), stop=(dc == CH - 1))
            nc.vector.tensor_copy(t_xkc[:, b, :], ps[:])

        # ---------------- recurrent state ----------------
        state = []   # [8,28] b-major per group
        stateT = []  # [28,8] per group
        for g in range(G):
            sg = wrk.tile([BG, V], f32, tag=f"state{g}")
            sTg = wrk.tile([V, BG], f32, tag=f"stateT{g}")
            nc.vector.memset(sg[:], 0.0)
            nc.vector.memset(sTg[:], 0.0)
            state.append(sg)
            stateT.append(sTg)

        # ---------------- the 75 steps ----------------
        has_gp_tt = hasattr(nc.gpsimd, "tensor_tensor")
        for s in range(steps):
            new_state, new_stateT = [], []
            bt_list, wast_list, tanhY = [], [], None
            tanhY = wrk.tile([NC_, CH, BL, T], f32, tag="tanhY")
            # --- phase 1 (per group): state matmuls, bias-add, tanh ---
            for g in range(G):
                sg, sTg = state[g], stateT[g]
                bs = g * BG
                wast_ps = pwast.tile([NC_, 40], f32, tag="wastps")
                bt_ps = pbt.tile([BG, 256], f32, tag="btps")
                wast_list.append(wast_ps)
                bt_list.append(bt_ps)
                if s > 0:
                    for c in range(CH):
                        nc.tensor.matmul(wast_ps[:, c * 8:(c + 1) * 8],
                                         t_wa[:, c * 128:(c + 1) * 128], sTg[:],
                                         start=True, stop=True)
                nc.tensor.matmul(bt_ps[:, 0:84], sTg[:], t_wrec[:],
                                 start=True, stop=False)
                nc.tensor.matmul(bt_ps[:, 0:84], t_o116[:, bs:bs + BG], t_b1[:],
                                 start=False, stop=True)
                nc.tensor.matmul(bt_ps[:, 84:112], sTg[:], t_uo[:],
                                 start=True, stop=True)

                ty_out = tanhY[:, :, bs:bs + BG, :]
                if s > 0:
                    wast_sb = wrk.tile([NC_, 32], f32, tag=f"wastsb{g}")
                    nc.vector.tensor_copy(wast_sb[:], wast_ps[:, 0:32])
                    Yg = wrk.tile([NC_, CH, BG, T], f32, tag=f"Y{g}")

                    def bias_add(eng, c0, cn):
                        w_sl = wast_sb[:, c0 * 8:(c0 + cn) * 8]
                        w_bc = bass.AP(
                            w_sl.tensor, w_sl.offset,
                            [list(w_sl.ap[0]), [8, cn], [1, 8], [0, T]])
                        eng.tensor_tensor(Yg[:, c0:c0 + cn, :, :],
                                          t_uahp[:, c0:c0 + cn, bs:bs + BG, :],
                                          w_bc, Op.add)
                    if g == 1 and has_gp_tt:
                        bias_add(nc.vector, 0, 2)
                        bias_add(nc.gpsimd, 2, 2)
                    else:
                        bias_add(nc.vector, 0, CH)
                    nc.scalar.activation(ty_out, Yg[:], Act.Tanh)
                else:
                    nc.scalar.activation(ty_out, t_uahp[:, :, bs:bs + BG, :],
                                         Act.Tanh)

            # --- phase 2 (shared): scoresT via REP + per-b diag STT, exp ---
            scT = wrk.tile([T, BL], f32, tag="scT")
            junk = wrk.tile([T, T], f32, tag="junk")
            for b0, nb in SUBS:
                rep_ps = prep.tile([T, 6 * T], f32, tag="prep")
                rep = rep_ps[:].rearrange("p (b t) -> p b t", b=6)[:, 0:nb, :]
                for c in range(CH):
                    nc.tensor.matmul(rep, t_vas[:, c, :],
                                     tanhY[:, c, b0:b0 + nb, :],
                                     start=(c == 0), stop=(c == CH - 1))
                for k in range(nb):
                    nc.vector.scalar_tensor_tensor(
                        junk[:], rep[:, k, :], 1.0, t_diag[:],
                        Op.mult, Op.mult,
                        accum_out=scT[:, b0 + k:b0 + k + 1])
            expT = wrk.tile([T, BL], f32, tag="expT")
            nc.scalar.activation(expT[:], scT[:], Act.Exp)

            # --- phase 3 (per group): Z, SmBlk, xm/CoC, gates, pred ---
            for g in range(G):
                sg, sTg = state[g], stateT[g]
                bs = g * BG
                wast_ps, bt_ps = wast_list[g], bt_list[g]
                hm_sb = wrk.tile([BG, 84], f32, tag=f"hm{g}")
                nc.vector.tensor_copy(hm_sb[:], bt_ps[:, 0:84])
                nc.tensor.matmul(bt_ps[:, 112:113], expT[:, bs:bs + BG],
                                 t_two[:], start=True, stop=True)
                rhalf = wrk.tile([BG, 1], f32, tag=f"rhalf{g}")
                nc.vector.reciprocal(rhalf[:], bt_ps[:, 112:113])
                rfull = wrk.tile([BG, 1], f32, tag=f"rfull{g}")
                nc.vector.tensor_scalar(rfull[:], rhalf[:], 2.0, None, Op.mult)
                smb_dst = bass.AP(t_smb.tensor, t_smb[:].offset + 17 * bs,
                                  [list(t_smb[:].ap[0]), [17, BG]])
                nc.vector.tensor_copy(smb_dst, expT[:, bs:bs + BG])

                xm_ps = pxm.tile([BG, 112], f32, tag="pxm")
                for bl in range(BG):
                    b = bs + bl
                    nc.tensor.matmul(xm_ps[:], t_smb[:, 16 * b + bs:16 * b + bs + BG],
                                     t_xkc[:, b, :],
                                     start=(bl == 0), stop=(bl == BG - 1))

                # --- GRU gates ---
                zr = wrk.tile([BG, 56], f32, tag=f"zr{g}")
                nc.vector.scalar_tensor_tensor(zr[:], xm_ps[:, 0:56], rhalf[:],
                                               hm_sb[:, 0:56], Op.mult, Op.add)
                if gru_b0_nonzero:
                    nc.vector.tensor_tensor(zr[:], zr[:], t_b0[:, 0:56], Op.add)
                tz = wrk.tile([BG, 56], f32, tag=f"tz{g}")
                nc.scalar.activation(tz[:], zr[:], Act.Tanh)
                s1 = wrk.tile([BG, V], f32, tag=f"s1{g}")
                nc.vector.scalar_tensor_tensor(s1[:], tz[:, V:56], 1.0,
                                               hm_sb[:, 56:84], Op.add, Op.mult)
                ah = wrk.tile([BG, V], f32, tag=f"ah{g}")
                nc.vector.scalar_tensor_tensor(ah[:], xm_ps[:, 56:84], rfull[:],
                                               s1[:], Op.mult, Op.add)
                if gru_b0_nonzero:
                    nc.vector.tensor_tensor(ah[:], ah[:], t_b0[:, 56:84], Op.add)
                hh = wrk.tile([BG, V], f32, tag=f"hh{g}")
                nc.scalar.activation(hh[:], ah[:], Act.Tanh)
                d1 = wrk.tile([BG, V], f32, tag=f"d1{g}")
                nc.vector.tensor_sub(d1[:], sg[:], hh[:])
                d2 = wrk.tile([BG, V], f32, tag=f"d2{g}")
                nc.vector.tensor_add(d2[:], sg[:], hh[:])
                m1 = wrk.tile([BG, V], f32, tag=f"m1{g}")
                nc.vector.tensor_mul(m1[:], tz[:, 0:V], d1[:])
                ns = wrk.tile([BG, V], f32, tag=f"state{g}")
                nc.vector.tensor_tensor(ns[:], m1[:], d2[:], Op.add)
                nc.vector.tensor_scalar(ns[:], ns[:], 0.5, None, Op.mult)

                # --- stateT for next step ---
                nc.tensor.transpose(wast_ps[0:V, 32:40], ns[:], t_i16[0:BG, 0:BG])
                nsT = wrk.tile([V, BG], f32, tag=f"stateT{g}")
                nc.vector.tensor_copy(nsT[:], wast_ps[0:V, 32:40])

                # --- pred logits + softmax (this step's output) ---
                l1 = wrk.tile([BG, V], f32, tag=f"l1{g}")
                if s > 0:
                    ind = wrk.tile([BG, V], f32, tag=f"ind{g}")
                    nc.vector.tensor_scalar(ind[:], t_out[g][:, s - 1, :], 1.0,
                                            None, Op.is_ge)
                    l2 = wrk.tile([BG, V], f32, tag=f"l2{g}")
                    nc.vector.scalar_tensor_tensor(l2[:], ind[:], t_dw[:],
                                                   t_bo2[:], Op.mult, Op.add)
                    nc.vector.scalar_tensor_tensor(
                        l1[:], xm_ps[:, 84:112], rfull[:], l2[:],
                        Op.mult, Op.add)
                else:
                    nc.vector.scalar_tensor_tensor(
                        l1[:], xm_ps[:, 84:112], rfull[:], t_bo2[:],
                        Op.mult, Op.add)
                logits = wrk.tile([BG, V], f32, tag=f"logits{g}")
                nc.vector.tensor_tensor(logits[:], l1[:], bt_ps[:, 84:112],
                                        Op.add)
                expP = wrk.tile([BG, V], f32, tag=f"expP{g}")
                zp = wrk.tile([BG, 1], f32, tag=f"zp{g}")
                nc.scalar.activation(expP[:], logits[:], Act.Exp,
                                     accum_out=zp[:])
                rp = wrk.tile([BG, 1], f32, tag=f"rp{g}")
                nc.vector.reciprocal(rp[:], zp[:])
                nc.vector.tensor_scalar(t_out[g][:, s, :], expP[:], rp[:],
                                        None, Op.mult)

                new_state.append(ns)
                new_stateT.append(nsT)

            state, stateT = new_state, new_stateT

        for g in range(G):
            nc.sync.dma_start(y_out[g * BG:(g + 1) * BG, :, :], t_out[g][:])
    return dr, y_out


_CACHE = {}


def _get_program(gru_b0_nonzero, steps=T):
    key = (bool(gru_b0_nonzero), steps)
    if key in _CACHE:
        return _CACHE[key]
    import concourse.bass as bass
    import concourse.bacc as bacc
    import concourse.tile as tile
    from concourse import mybir

    nc = bacc.Bacc("TRN2", target_bir_lowering=False, debug=False,
                   num_devices=NCORES)
    with tile.TileContext(nc) as tc:
        _build(nc, tc, tile, bass, mybir, gru_b0_nonzero, steps)
    nc.compile()
    _CACHE[key] = nc
    return nc


def _prep_core_inputs(inputs, core):
    x = inputs["x"]
    xs = np.ascontiguousarray(x[core * BL:(core + 1) * BL]).astype(np.float32)
    # [16,75,512] -> [128, chunk, b, t]
    x_dmaj = np.ascontiguousarray(
        xs.reshape(BL, T, CH, 128).transpose(3, 2, 0, 1))
    return x_dmaj


def _prep_weights(inputs):
    f = np.float32
    Ua = inputs["Ua"].astype(f)
    ua_k = np.ascontiguousarray(
        Ua.reshape(CH, 128, CH, 128).transpose(1, 0, 2, 3))
    ba = (inputs["Ba1"] + inputs["Ba2"]).astype(f).reshape(CH, 128)
    ba12 = np.ascontiguousarray(ba.T)
    Va = inputs["Va"].astype(f).reshape(CH, 128)
    vasel = np.ascontiguousarray(
        np.repeat(Va.T[:, :, None], T, axis=2))
    w2 = np.concatenate([inputs["gru_kernel"], inputs["Co"]], axis=1).astype(f)
    w2 = np.ascontiguousarray(w2.reshape(CH, 128, 112).transpose(1, 0, 2))
    w = (inputs["emb"].astype(f) @ inputs["Wo"].astype(f)).reshape(-1)
    w0, w1 = float(w[0]), float(w[1])
    gb = inputs["gru_bias"].astype(f)
    out = {
        "ua_k": ua_k, "ba12": ba12, "wa": inputs["Wa"].astype(f),
        "vasel": vasel, "w2": w2,
        "wrec_h": 0.5 * inputs["gru_rec_kernel"].astype(f),
        "b1gru": gb[1:2], "uo": inputs["Uo"].astype(f),
        "diag": np.eye(T, dtype=f),
        "twos75": np.full([T, 1], 2.0, dtype=f),
        "ones1x16": np.ones([1, BL], dtype=f),
        "i16": np.eye(BL, dtype=f),
        "bo2rep": np.repeat(inputs["Bo"].astype(f) + w0, BG, axis=0),
        "dwrep": np.full([BG, 1], w1 - w0, dtype=f),
    }
    b0 = gb[0]
    if np.any(b0 != 0):
        out["b0rep"] = np.repeat(b0[None, :], BG, axis=0)
    return out, bool(np.any(b0 != 0))


def kernel(**inputs):
    from concourse.bass_utils import run_bass_kernel_spmd

    weights, b0nz = _prep_weights(inputs)
    nc = _get_program(b0nz)
    in_maps = []
    for core in range(NCORES):
        m = dict(weights)
        m["x_dmaj"] = _prep_core_inputs(inputs, core)
        in_maps.append(m)
    res = run_bass_kernel_spmd(nc, in_maps, core_ids=list(range(NCORES)))
    out = np.concatenate([res.results[c]["y"] for c in range(NCORES)], axis=0)
    return out.astype(np.float32)



